# revision 1
# baseline (speedup 1.0000x reference)
"""Causal self-attention (B=4, T=2048, C=1024, H=16, D=64) on 8 trn2 NeuronCores.

Sharding: core c handles batch b = c//2 and head group g = c%2 (8 heads each).
Each core computes the qkv projection for its heads, causal flash attention,
and a partial output projection (its heads' rows of w_proj). The host sums the
two partials per batch.

Per-core kernel layout notes:
  - The qkv projection runs in fp8(e4m3) DoubleRow perf mode with an exact-ish
    hi/lo split: x = xh + xl, w = wh + wl (host-precomputed, w pre-scaled by
    32 so its lo part clears the fp8 subnormal floor). x@w is computed as
    xh@wh + xl@wh + xh@wl (the dropped xl@wl term is ~0.1% relative).
    DoubleRow packs two 128-row contraction blocks per instruction
    ([128,2,M] stationary / [128,2,N] moving) at half the bf16 row cost, so
    the projection is 0.75x its bf16 version.
  - Inputs live in single big SBUF tiles laid out partition-major on the host
    ([128, 4 kpair, 2 krow, cols]; w_attn columns re-ordered q0 k0 .. q3 k3 v)
    so DMA arrives in consumption-priority chunks: pair-0 q/k weights, x
    q-block 0, v weights, then later q blocks; the first attention block can
    start ~10us in while the rest streams.
  - The 1/32 weight prescale is folded into the PSUM->SBUF copies of q/k/v
    (ScalarE Copy-with-scale); the attention core is unchanged bf16:
    Q^T/K^T head-pair-packed tiles [128, T], S^T tiles get exp on ScalarE
    with the 1/sqrt(D)=0.125 scale folded in, softmax denominator via a
    0.125-valued ones-column on V (M=65 AV matmul, so rec = 8/sum).
  - The softmax reciprocal row is broadcast with a PE ones-matmul (the only
    HW-safe partition broadcast); both reciprocals are issued first with a
    reserved filler chunk between them and the broadcasts so the PE never
    head-of-line stalls on DVE. y^T (x8 scaled) is divided before the bf16
    projection; w_proj carries 1/8 so output copies are plain. Output DMAs
    in bf16; the host upcasts and sums the two partials.
  - Work order: pair-0 Q/K -> V -> per-pair attention with qkv/proj closures
    front-loaded into the attention kt loops so the PE static order never
    runs dry while ScalarE paces exp; the final projection group is split so
    only its pair-3 accumulation trails the last division.
"""

import sys

sys.path.insert(0, "/opt/trn_rl_repo")

import numpy as np
import ml_dtypes

from concourse import bacc, mybir
import concourse.bass as bass
from concourse.tile import TileContext
from concourse.bass_utils import run_bass_kernel_spmd

B, T, C, H, D = 4, 2048, 1024, 16, 64
N_CORES = 8
HL = H // 2  # heads per core: 8
CL = HL * D  # local channels per core: 512
BF16 = mybir.dt.bfloat16
F32 = mybir.dt.float32
FP8 = mybir.dt.float8e4
DR = mybir.MatmulPerfMode.DoubleRow
TT_TILES = T // 128  # 16 token tiles
QB = 4  # q blocks of 512
WSCALE = 32.0  # host prescale on w_attn (folded out on-chip)
YSCALE = 8.0  # yt carries 8*y/denom (ones column = 0.125)


def build_program(unroll=1, diag_restrict=True, proj_interleave=True, tb_fuse=True):
    """unroll>1 repeats the whole compute body (for steady-state timing)."""
    nc = bacc.Bacc("TRN2", target_bir_lowering=False, debug=False, num_devices=N_CORES)
    xh = nc.dram_tensor("xh", [128, 4, 2, T], FP8, kind="ExternalInput")
    xl = nc.dram_tensor("xl", [128, 4, 2, T], FP8, kind="ExternalInput")
    wah = nc.dram_tensor("wah", [128, 4, 2, 3 * CL], FP8, kind="ExternalInput")
    wal = nc.dram_tensor("wal", [128, 4, 2, 3 * CL], FP8, kind="ExternalInput")
    wp = nc.dram_tensor("wp", [128, 4, C], BF16, kind="ExternalInput")
    wpb = nc.dram_tensor("wpb", [64, C], BF16, kind="ExternalInput")
    msk = nc.dram_tensor("msk", [128, 128], BF16, kind="ExternalInput")
    outp = nc.dram_tensor("outp", [T, C], BF16, kind="ExternalOutput")

    with TileContext(nc) as tc:
        with tc.tile_pool(name="resid", bufs=1) as p_r, tc.tile_pool(
            name="ps", bufs=1, space="PSUM"
        ) as p_ps, tc.tile_pool(name="es", bufs=3) as p_es, tc.tile_pool(
            name="sc", bufs=4
        ) as p_sc, tc.tile_pool(name="ob", bufs=3) as p_ob:
            xh_sb = p_r.tile([128, 4, 2, T], FP8, tag="xh")
            xl_sb = p_r.tile([128, 4, 2, T], FP8, tag="xl")
            wah_sb = p_r.tile([128, 4, 2, 3 * CL], FP8, tag="wah")
            wal_sb = p_r.tile([128, 4, 2, 3 * CL], FP8, tag="wal")
            wp_sb = p_r.tile([128, 4, C], BF16, tag="wp")
            wpb_sb = p_r.tile([64, C], BF16, tag="wpb")
            msk_sb = p_r.tile([128, 128], BF16, tag="msk")
            ones_sb = p_r.tile([128, 64], BF16, tag="ones")
            # bf16 partial sums of the final projection group (pairs 0-2),
            # closed early so the PSUM fill ring keeps cycling.
            fp_sb = [
                [p_r.tile([128, 512], BF16, name=f"fp{u}{nb}", tag=f"fp{u}{nb}") for nb in range(2)]
                for u in range(4)
            ]
            qt_sb = [p_r.tile([128, T], BF16, name=f"qt{p}", tag=f"qt{p}") for p in range(4)]
            kt_sb = [p_r.tile([128, T], BF16, name=f"kt{p}", tag=f"kt{p}") for p in range(4)]
            va_sb = [p_r.tile([128, HL * 65], BF16, name=f"va{i}", tag=f"va{i}") for i in range(TT_TILES)]
            yt_sb = [p_r.tile([128, T], BF16, name=f"yt{p}", tag=f"yt{p}") for p in range(4)]

            def XH(j, a, b):
                return xh_sb[:, j : j + 1, :, a:b].squeeze(1)

            def XL(j, a, b):
                return xl_sb[:, j : j + 1, :, a:b].squeeze(1)

            def WAH(j, a, b):
                return wah_sb[:, j : j + 1, :, a:b].squeeze(1)

            def WAL(j, a, b):
                return wal_sb[:, j : j + 1, :, a:b].squeeze(1)

            nc.sync.dma_start(out=msk_sb[:], in_=msk[:])

            # HAM warmup: the PE clock-gate sits at 1.2 GHz until ~3.4us of
            # sustained activity; burn the initial DMA window with dummy
            # matmuls on a memset tile (borrowing an "av" PSUM slot).
            warm = p_r.tile([128, 512], BF16, tag="warm")
            nc.gpsimd.memset(warm[:], 0.0)
            nc.gpsimd.memset(ones_sb[:], 1.0)
            wps = p_ps.tile([128, 512], F32, tag="av", name="warmps", bufs=2)
            for _ in range(6):
                nc.tensor.matmul(
                    wps[:], lhsT=warm[:, 0:128], rhs=warm[:], start=True, stop=True
                )

            # DMA in consumption-priority chunks (transfer order == issue
            # order): pair-0 q/k weights + x q-block 0 + v weights unblock
            # the first attention block ~10us in; later q blocks and the
            # remaining pairs' weight columns stream in behind.
            def dma_cols(dst, src, a, b):
                nc.sync.dma_start(out=dst[:, :, :, a:b], in_=src[:, :, :, a:b])

            dma_cols(wah_sb, wah, 0, 256)
            dma_cols(xh_sb, xh, 0, 512)
            dma_cols(xl_sb, xl, 0, 512)
            dma_cols(wal_sb, wal, 0, 256)
            dma_cols(wah_sb, wah, 2 * CL, 3 * CL)
            dma_cols(wal_sb, wal, 2 * CL, 3 * CL)
            for qb in range(1, QB):
                dma_cols(xh_sb, xh, qb * 512, (qb + 1) * 512)
                dma_cols(xl_sb, xl, qb * 512, (qb + 1) * 512)
            dma_cols(wah_sb, wah, 256, 1024)
            dma_cols(wal_sb, wal, 256, 1024)
            nc.sync.dma_start(out=wp_sb[:], in_=wp[:])
            nc.sync.dma_start(out=wpb_sb[:], in_=wpb[:])

            for _ in range(unroll):
                # ---- closure builders; each closure is a chunk of PE work
                # that can be interleaved into the attention kt loops so the
                # PE static order never runs dry while ScalarE paces exp.
                def fp8_group(acc, lhs_hi, lhs_lo, rhs_hi, rhs_lo):
                    """The 12-instruction hi/lo DoubleRow group as two
                    chunks: [T1-hi] and [T1-lo + T2]."""

                    def part1():
                        for j in range(4):
                            nc.tensor.matmul(
                                acc[:],
                                lhsT=lhs_hi[j],
                                rhs=rhs_hi[j],
                                start=(j == 0),
                                stop=False,
                                perf_mode=DR,
                            )

                    def part2():
                        for j in range(4):
                            nc.tensor.matmul(
                                acc[:],
                                lhsT=lhs_lo[j],
                                rhs=rhs_hi[j],
                                start=False,
                                stop=False,
                                perf_mode=DR,
                            )
                        for j in range(4):
                            nc.tensor.matmul(
                                acc[:],
                                lhsT=lhs_hi[j],
                                rhs=rhs_lo[j],
                                start=False,
                                stop=(j == 3),
                                perf_mode=DR,
                            )

                    return part1, part2

                def v_closures(tt):
                    state = {}

                    def h1():
                        acc = p_ps.tile([128, 512], F32, tag="fill", name="accv", bufs=2)
                        state["acc"] = acc
                        state["p1"], state["p2"] = fp8_group(
                            acc,
                            [XH(j, tt * 128, (tt + 1) * 128) for j in range(4)],
                            [XL(j, tt * 128, (tt + 1) * 128) for j in range(4)],
                            [WAH(j, 2 * CL, 3 * CL) for j in range(4)],
                            [WAL(j, 2 * CL, 3 * CL) for j in range(4)],
                        )
                        state["p1"]()

                    def h2():
                        acc = state["acc"]
                        state["p2"]()
                        va_view = va_sb[tt].rearrange("p (h e) -> p h e", e=65)
                        nc.scalar.activation(
                            out=va_view[:, :, 0:64],
                            in_=acc[:].rearrange("p (h e) -> p h e", e=64),
                            func=mybir.ActivationFunctionType.Copy,
                            scale=1.0 / WSCALE,
                        )
                        nc.gpsimd.memset(va_view[:, :, 64:65], 1.0 / YSCALE)

                    return [h1, h2]

                def accqk_closures(p, qb):
                    out = []
                    for dst, col0 in ((qt_sb, 256 * p), (kt_sb, 256 * p + 128)):
                        state = {}

                        def h1(dst=dst, col0=col0, state=state):
                            acc = p_ps.tile([128, 512], F32, tag="fill", name="accqk", bufs=2)
                            state["acc"] = acc
                            state["p1"], state["p2"] = fp8_group(
                                acc,
                                [WAH(j, col0, col0 + 128) for j in range(4)],
                                [WAL(j, col0, col0 + 128) for j in range(4)],
                                [XH(j, qb * 512, (qb + 1) * 512) for j in range(4)],
                                [XL(j, qb * 512, (qb + 1) * 512) for j in range(4)],
                            )
                            state["p1"]()

                        def h2(dst=dst, col0=col0, state=state):
                            state["p2"]()
                            nc.scalar.activation(
                                out=dst[p][:, qb * 512 : (qb + 1) * 512],
                                in_=state["acc"][:],
                                func=mybir.ActivationFunctionType.Copy,
                                scale=1.0 / WSCALE,
                            )

                        out += [h1, h2]
                    return out

                def proj_closures(tt):
                    state = {}

                    def mk(nb):
                        def h():
                            if nb == 0:
                                state["ob"] = p_ob.tile([128, C], BF16, tag="ob", name="ob")
                            acc = p_ps.tile(
                                [128, 512], F32, tag="fill", name="accp", bufs=2
                            )
                            for pp in range(4):
                                nc.tensor.matmul(
                                    acc[:],
                                    lhsT=yt_sb[pp][:, tt * 128 : (tt + 1) * 128],
                                    rhs=wp_sb[:, pp : pp + 1, nb * 512 : (nb + 1) * 512],
                                    start=(pp == 0),
                                    stop=(pp == 3),
                                )
                            ob = state["ob"]
                            nc.vector.tensor_copy(
                                out=ob[:, nb * 512 : (nb + 1) * 512], in_=acc[:]
                            )
                            nc.sync.dma_start(
                                out=outp[
                                    tt * 128 : (tt + 1) * 128, nb * 512 : (nb + 1) * 512
                                ],
                                in_=ob[:, nb * 512 : (nb + 1) * 512],
                            )

                        return h

                    return [mk(0), mk(1)]

                def final_early_closures(tt):
                    # pairs 0-2 of the last projection group: closed psum
                    # groups copied to bf16 partials so no bank stays open
                    # across the last division chain.
                    u = tt - 4 * (QB - 1)

                    def mk(nb):
                        def h():
                            acc = p_ps.tile(
                                [128, 512], F32, tag="fill", name="accf", bufs=2
                            )
                            for pp in range(3):
                                nc.tensor.matmul(
                                    acc[:],
                                    lhsT=yt_sb[pp][:, tt * 128 : (tt + 1) * 128],
                                    rhs=wp_sb[:, pp : pp + 1, nb * 512 : (nb + 1) * 512],
                                    start=(pp == 0),
                                    stop=(pp == 2),
                                )
                            nc.vector.tensor_copy(out=fp_sb[u][nb][:], in_=acc[:])

                        return h

                    return [mk(0), mk(1)]

                def final_tail_closures(tt, tmpB):
                    # pair-3 adds: head A from yt (DVE-written), head B read
                    # straight from the division scratch (no DMA hop), then
                    # ob = pair3 + partial and store.
                    u = tt - 4 * (QB - 1)
                    state = {}

                    def mk(nb):
                        def h():
                            if nb == 0:
                                state["ob"] = p_ob.tile([128, C], BF16, tag="ob", name="ob")
                            acc = p_ps.tile(
                                [128, 512], F32, tag="fill", name="acct", bufs=2
                            )
                            nc.tensor.matmul(
                                acc[:],
                                lhsT=yt_sb[3][0:64, tt * 128 : (tt + 1) * 128],
                                rhs=wp_sb[0:64, 3:4, nb * 512 : (nb + 1) * 512],
                                start=True,
                                stop=False,
                            )
                            nc.tensor.matmul(
                                acc[:],
                                lhsT=tmpB[0:64, u * 128 : (u + 1) * 128],
                                rhs=wpb_sb[:, nb * 512 : (nb + 1) * 512],
                                start=False,
                                stop=True,
                            )
                            ob = state["ob"]
                            nc.vector.tensor_tensor(
                                ob[:, nb * 512 : (nb + 1) * 512],
                                acc[:],
                                fp_sb[u][nb][:],
                                mybir.AluOpType.add,
                            )
                            nc.sync.dma_start(
                                out=outp[
                                    tt * 128 : (tt + 1) * 128, nb * 512 : (nb + 1) * 512
                                ],
                                in_=ob[:, nb * 512 : (nb + 1) * 512],
                            )

                        return h

                    return [mk(0), mk(1)]

                blocks = [(p, qb) for p in range(4) for qb in range(QB)]

                # prologue: pair 0's first QT/KT (fed by the earliest DMAs),
                # then V tiles for q block 0
                for cl in accqk_closures(0, 0):
                    cl()
                for tt in range(4):
                    for cl in v_closures(tt):
                        cl()

                last_tmpB = [None]
                for idx, (p, qb) in enumerate(blocks):
                    hA, hB = 2 * p, 2 * p + 1
                    filler = []
                    if idx + 1 < len(blocks):
                        filler += accqk_closures(*blocks[idx + 1])
                    if p == 0 and qb < QB - 1:
                        for tt in range(4 * (qb + 1), 4 * (qb + 1) + 4):
                            filler += v_closures(tt)
                    if p == 3 and qb > 0 and proj_interleave:
                        for tt in range(4 * (qb - 1), 4 * qb):
                            filler += proj_closures(tt)
                    if idx >= len(blocks) - 2 and proj_interleave:
                        # pairs 0-2 of the final projection group ride along
                        # in the last two blocks (closed groups + partials).
                        lo = 4 * (QB - 1) + 2 * (idx - (len(blocks) - 2))
                        for tt in range(lo, lo + 2):
                            filler += final_early_closures(tt)

                    # Keep a few PE-heavy chunks (with no tail consumers) in
                    # reserve so the PE has work to chew while the final
                    # division chain resolves. Front of the list = plain
                    # projection groups whose copies gate nothing downstream.
                    hold = []
                    if idx == len(blocks) - 1 and len(filler) > 2:
                        # even count only: proj closures come in mk0/mk1
                        # pairs sharing an ob tile
                        hold = filler[0:2]
                        del filler[0:2]
                    div_hold = []
                    if len(filler) > 3:
                        div_hold = filler[-3:]
                        del filler[-3:]

                    filler_all = list(filler)
                    nkt = 4 * qb + 4
                    LAG = 5  # AV trails QK by this many k tiles
                    ya = p_ps.tile([128, 512], F32, tag="av", name="ya", bufs=2)
                    yb = p_ps.tile([128, 512], F32, tag="av", name="yb", bufs=2)
                    es_ring = {}

                    def emit_qk(kt, p=p, qb=qb):
                        # Diagonal tiles only need q columns >= kt*128; the
                        # causal triangle then only lives in the first 128 of
                        # the remaining columns. st/es keep head B at column
                        # 512 (fixed full-size tiles).
                        j = kt - 4 * qb
                        qoff = max(j, 0) * 128 if diag_restrict else 0
                        W = 512 - qoff
                        st = p_ps.tile([128, 1024], F32, tag="st", name="st", bufs=2)
                        nc.tensor.matmul(
                            st[:, 0:W],
                            lhsT=kt_sb[p][0:64, kt * 128 : (kt + 1) * 128],
                            rhs=qt_sb[p][0:64, qb * 512 + qoff : (qb + 1) * 512],
                            start=True,
                            stop=True,
                        )
                        nc.tensor.matmul(
                            st[:, 512 : 512 + W],
                            lhsT=kt_sb[p][64:128, kt * 128 : (kt + 1) * 128],
                            rhs=qt_sb[p][64:128, qb * 512 + qoff : (qb + 1) * 512],
                            start=True,
                            stop=True,
                        )
                        es = p_es.tile([128, 1024], BF16, tag="es", name="es", bufs=8)
                        if W == 512:
                            nc.scalar.activation(
                                out=es[:],
                                in_=st[:],
                                func=mybir.ActivationFunctionType.Exp,
                                scale=0.125,
                            )
                        else:
                            view_es = es.rearrange("p (h w) -> p h w", h=2)[:, :, 0:W]
                            view_st = st.rearrange("p (h w) -> p h w", h=2)[:, :, 0:W]
                            nc.scalar.activation(
                                out=view_es,
                                in_=view_st,
                                func=mybir.ActivationFunctionType.Exp,
                                scale=0.125,
                            )
                        if j >= 0:
                            nc.vector.tensor_tensor(
                                es[:, 0:128], es[:, 0:128], msk_sb[:], mybir.AluOpType.mult
                            )
                            nc.vector.tensor_tensor(
                                es[:, 512:640], es[:, 512:640], msk_sb[:], mybir.AluOpType.mult
                            )
                        es_ring[kt] = (es, qoff, W)

                    def emit_av(kt, p=p, qb=qb, nkt=nkt, ya=ya, yb=yb):
                        es, qoff, W = es_ring.pop(kt)
                        nc.tensor.matmul(
                            ya[0:65, qoff:512],
                            lhsT=va_sb[kt][:, hA * 65 : (hA + 1) * 65],
                            rhs=es[:, 0:W],
                            start=(kt == 0),
                            stop=(kt == nkt - 1),
                        )
                        nc.tensor.matmul(
                            yb[0:65, qoff:512],
                            lhsT=va_sb[kt][:, hB * 65 : (hB + 1) * 65],
                            rhs=es[:, 512 : 512 + W],
                            start=(kt == 0),
                            stop=(kt == nkt - 1),
                        )

                    total_iters = nkt + LAG
                    pops_done = 0
                    for kt in range(total_iters):
                        if kt < nkt:
                            emit_qk(kt)
                        if kt >= LAG:
                            emit_av(kt - LAG)
                        # front-loaded, with two pops up front: the st ring
                        # (2 banks) stalls the third QK tile until the first
                        # exp completes, so give the PE filler to chew there.
                        want = min(
                            len(filler_all),
                            max(2, 2 + kt * len(filler_all) // max(nkt - 2, 1)),
                        )
                        while pops_done < want and filler:
                            filler.pop(0)()
                            pops_done += 1
                    while filler:
                        filler.pop(0)()

                    div_order = ((0, ya), (1, yb))
                    # both reciprocals first, then a reserved filler chunk on
                    # the PE so the ones-matmul broadcasts don't head-of-line
                    # stall the PE while the reciprocals complete on DVE
                    recs = {}
                    for s, yy in div_order:
                        recs[s] = p_sc.tile([128, 512], BF16, tag="rec", name="rec")
                        with nc.allow_low_precision(reason="softmax denom in bf16"):
                            nc.vector.reciprocal(out=recs[s][64:65, :], in_=yy[64:65, :])
                    while div_hold:
                        div_hold.pop(0)()
                    for s, yy in div_order:
                        rec = recs[s]
                        reps = p_sc.tile([128, 512], BF16, tag="reps", name="reps")
                        rep = p_ps.tile([128, 512], F32, tag="fill", name="rep", bufs=2)
                        nc.tensor.matmul(
                            rep[0:64, :],
                            lhsT=ones_sb[64:65, 0:64],
                            rhs=rec[64:65, :],
                            start=True,
                            stop=True,
                        )
                        nc.vector.tensor_copy(out=reps[0:64, :], in_=rep[0:64, :])
                        if s == 0:
                            nc.vector.tensor_tensor(
                                yt_sb[p][0:64, qb * 512 : (qb + 1) * 512],
                                yy[0:64, :],
                                reps[0:64, :],
                                mybir.AluOpType.mult,
                            )
                        else:
                            tmp = p_sc.tile([128, 512], BF16, tag="ytmp", name="ytmp")
                            nc.vector.tensor_tensor(
                                tmp[0:64, :],
                                yy[0:64, :],
                                reps[0:64, :],
                                mybir.AluOpType.mult,
                            )
                            if idx == len(blocks) - 1 and proj_interleave:
                                # the final tails read head B straight from
                                # this scratch; no partition-shift DMA needed
                                last_tmpB[0] = tmp
                            else:
                                nc.sync.dma_start(
                                    out=yt_sb[p][64:128, qb * 512 : (qb + 1) * 512],
                                    in_=tmp[0:64, :],
                                )

                # reserved work overlaps the final division chain, then the
                # trailing pair-3 adds + stores of the final projection group
                if proj_interleave:
                    for cl in hold:
                        cl()
                    for tt in range(4 * (QB - 1), 4 * QB):
                        for cl in final_tail_closures(tt, last_tmpB[0]):
                            cl()
                else:
                    for tt in range(4 * (QB - 1), 4 * QB):
                        for cl in proj_closures(tt):
                            cl()

    nc.compile()
    return nc


_NC_CACHE = None


def _get_program():
    global _NC_CACHE
    if _NC_CACHE is None:
        _NC_CACHE = build_program()
    return _NC_CACHE


def _make_masks():
    ki = np.arange(128)[:, None]
    qi = np.arange(128)[None, :]
    return (ki <= qi).astype(np.float32).astype(ml_dtypes.bfloat16)


_FP8_NP = ml_dtypes.float8_e4m3


def _fp8_hilo(a):
    """Split a float32 matrix into fp8 hi + lo with hi+lo ~= a."""
    hi = a.astype(_FP8_NP)
    lo = (a - hi.astype(np.float32)).astype(_FP8_NP)
    return hi, lo


def _pair_layout(a, ncols):
    """[1024, ncols] -> [128, 4, 2, ncols]: partition-major, 128-row blocks
    paired (2j, 2j+1) along the DoubleRow sub-row axis."""
    return np.ascontiguousarray(
        a.reshape(4, 2, 128, ncols).transpose(2, 0, 1, 3)
    )


def make_in_maps(x, w_attn, w_proj):
    msk = _make_masks()
    xhs, xls = [], []
    for b in range(B):
        xt = np.asarray(x[b], dtype=np.float32).T  # [C, T]
        hi, lo = _fp8_hilo(xt)
        xhs.append(_pair_layout(hi, T))
        xls.append(_pair_layout(lo, T))
    wahs, wals, wps = [], [], []
    for g in range(2):
        wq = w_attn[:, 512 * g : 512 * g + 512]
        wk = w_attn[:, C + 512 * g : C + 512 * g + 512]
        wv = w_attn[:, 2 * C + 512 * g : 2 * C + 512 * g + 512]
        # columns q0 k0 q1 k1 q2 k2 q3 k3 v (128 per q/k sub-block)
        qk_cols = []
        for p in range(4):
            qk_cols.append(wq[:, p * 128 : (p + 1) * 128])
            qk_cols.append(wk[:, p * 128 : (p + 1) * 128])
        wa_local = np.concatenate(qk_cols + [wv], axis=1).astype(np.float32) * WSCALE
        hi, lo = _fp8_hilo(wa_local)
        wahs.append(_pair_layout(hi, 3 * CL))
        wals.append(_pair_layout(lo, 3 * CL))
        # wp carries the 1/8 that cancels yt's x8, so projection accs come
        # out unscaled and every output copy is a plain copy/add.
        wp_f = (
            w_proj[512 * g : 512 * g + 512, :].astype(np.float32) / YSCALE
        ).astype(ml_dtypes.bfloat16)
        wps.append(
            np.ascontiguousarray(wp_f.reshape(4, 128, C).transpose(1, 0, 2))
        )
        wpbs_g = np.ascontiguousarray(wp_f[448:512, :])
        wps[-1] = (wps[-1], wpbs_g)
    return [
        {
            "xh": xhs[c // 2],
            "xl": xls[c // 2],
            "wah": wahs[c % 2],
            "wal": wals[c % 2],
            "wp": wps[c % 2][0],
            "wpb": wps[c % 2][1],
            "msk": msk,
        }
        for c in range(N_CORES)
    ]


def gather_output(results):
    out = np.empty((B, T, C), np.float32)
    for b in range(B):
        out[b] = results[2 * b]["outp"].astype(np.float32) + results[
            2 * b + 1
        ]["outp"].astype(np.float32)
    return out


_RUNNER = None


def _make_cached_runner(nc):
    """jit the SPMD executable once so repeat kernel() calls skip the
    per-call retrace/recompile that run_bass_kernel_spmd pays."""
    import jax
    from jax.sharding import Mesh, PartitionSpec
    from jax.experimental.shard_map import shard_map
    from concourse import bass2jax

    bass2jax.install_neuronx_cc_hook()
    partition_name = nc.partition_id_tensor.name if nc.partition_id_tensor else None
    in_names, out_names, out_avals, zero_outs = [], [], [], []
    for alloc in nc.m.functions[0].allocations:
        if not isinstance(alloc, mybir.MemoryLocationSet):
            continue
        name = alloc.memorylocations[0].name
        if alloc.kind == "ExternalInput":
            if name != partition_name:
                in_names.append(name)
        elif alloc.kind == "ExternalOutput":
            shape = tuple(alloc.tensor_shape)
            dtype = mybir.dt.np(alloc.dtype)
            out_names.append(name)
            out_avals.append(jax.core.ShapedArray(shape, dtype))
            zero_outs.append(np.zeros(shape, dtype))
    n_params = len(in_names)
    n_outs = len(out_avals)
    all_in_names = in_names + out_names
    if partition_name is not None:
        all_in_names.append(partition_name)

    def _body(*args):
        operands = list(args)
        if partition_name is not None:
            operands.append(bass2jax.partition_id_tensor())
        return tuple(
            bass2jax._bass_exec_p.bind(
                *operands,
                out_avals=tuple(out_avals),
                in_names=tuple(all_in_names),
                out_names=tuple(out_names),
                lowering_input_output_aliases=(),
                sim_require_finite=True,
                sim_require_nnan=True,
                nc=nc,
            )
        )

    devices = jax.devices()[:N_CORES]
    mesh = Mesh(np.asarray(devices), ("core",))
    spec = PartitionSpec("core")
    sharded = jax.jit(
        shard_map(
            _body,
            mesh=mesh,
            in_specs=(spec,) * (n_params + n_outs),
            out_specs=(spec,) * n_outs,
            check_rep=False,
        ),
        donate_argnums=tuple(range(n_params, n_params + n_outs)),
        keep_unused=True,
    )

    def run(in_maps):
        concat_in = [
            np.concatenate([np.asarray(in_maps[c][nm]) for c in range(N_CORES)], 0)
            for nm in in_names
        ]
        zeros = [
            np.zeros((N_CORES * z.shape[0], *z.shape[1:]), z.dtype) for z in zero_outs
        ]
        outs = sharded(*concat_in, *zeros)
        return [
            {
                name: np.asarray(outs[i]).reshape(N_CORES, *out_avals[i].shape)[c]
                for i, name in enumerate(out_names)
            }
            for c in range(N_CORES)
        ]

    return run


def kernel(x, w_attn, w_proj):
    global _RUNNER
    x = np.asarray(x, dtype=np.float32)
    w_attn = np.asarray(w_attn, dtype=np.float32)
    w_proj = np.asarray(w_proj, dtype=np.float32)
    nc = _get_program()
    in_maps = make_in_maps(x, w_attn, w_proj)
    if _RUNNER is None:
        try:
            _RUNNER = _make_cached_runner(nc)
        except Exception:
            _RUNNER = None
        if _RUNNER is None:
            res = run_bass_kernel_spmd(nc, in_maps, core_ids=list(range(N_CORES)))
            return gather_output(res.results)
    try:
        return gather_output(_RUNNER(in_maps))
    except Exception:
        res = run_bass_kernel_spmd(nc, in_maps, core_ids=list(range(N_CORES)))
        return gather_output(res.results)

